# revision 31
# baseline (speedup 1.0000x reference)
"""Autoregressive LSTM cell (B=256, T=256, D=256, H=1024, O=256) on 8 TRN2 cores.

Strategy: pure data-parallel over batch (32 rows/core, no collectives).
Per step t (sequential, 256 steps):
    z = x_t @ Wxx + y_{t-1} @ Wxy + h_{t-1} @ Wh     (+b)
    i,f,g,o gates -> c = sig(f)*c + sig(i)*tanh(g); h = sig(o)*tanh(c)
    y = tanh(h @ Wd + bd)
Matmuls are "activation-stationary": lhsT = activation^T [K<=128, M=32batch],
rhs streams bf16 weight columns at 1 column/cycle (fp32 PSUM accumulation;
fp32 gate math keeps the recurrence error ~1e-2-bounded).
4-way PE column tiling (tile_position col groups) packs 4 batch-32 matmuls
concurrently, writing z in a stacked PSUM layout:
  z_ps [128, 1024]: position (32j+b, 512*beta + n) = z_perm[2048*beta + 512j + n]
Weight columns are host-permuted so that
  bank0 (cols 0:512)  = [ sig-gate i | sig-gate f ] column-paired per channel
  bank1 (cols 512:1024) = [ tanh-gate g | sig-gate o ]
with channel(p=32j+b, n) = 256j + (n mod 256): all gate elementwise ops are
partition-aligned and the c update is a free-dim-shifted add.
h/y are fed back transposed via PE transpose-mode matmuls.

Overlap structure: the x-part matmuls of step t+1 are software-pipelined into
step t (the in-order PE would otherwise stall on the gate chain), z-chunks are
ordered x->h->y so the y-feedback chain hides under the h-matmul span, and the
gate chain runs in two column halves so each half's transpose/hT-copy releases
the even/odd Wh chunks of the next step independently.

Measured (axon, whole-body reps-differential: wall(reps=5)-wall(reps=1)
over device-resident re-dispatches): all-bf16 runs ~17.3 us/step (4.42 ms
total). Streaming the dominant Wh (8/12 K-chunks) as fp8e3m4 with x16
z-path pre-scaling (exact for the bf16 Wxx/Wxy; 1/16 folded into the gate
ACT scale) cuts the weight stream 12.6 -> 8.9 MB/step: measured ~11.4
us/step, 2.91 ms total. Precision ladder (measured end-to-end): all-bf16
7.77e-3; Wh-e3m4 1.58e-2 (PASS < 2e-2); Wh-e4m3 2.8e-2, +Wxy-e3m4
2.14e-2, x/y- or Wd-fp8 worse (x carries the largest z amplitude; Wd
hits the output directly) - all FAIL the 2e-2 gate.

Remaining bottleneck (measured via mm_only=1 with dependency-free dummy
feedback): the unconstrained PE stream is ~6.6 us/step, so ~4.8 us/step
of the 11.4 is the EXPOSED per-step serial chain (gate ACT/DVE ops +
feedback transposes + ~100ns/hop semaphore latency; the single ACT
engine serializes the 10 activations). x-prefetch fill is conserved
(one step of x per step regardless of depth), Pool-engine offload of
the DVE ops fails walrus codegen, and deeper gate splitting drowns in
per-op fixed overhead. The open fix: interleave two independent 16-row
half-batch recurrences offset by half a step so each group's matmuls
fill the PE during the other group's gate chain (est. ~2.0-2.3 ms).
"""

import sys

for p in ("/opt/trn_rl_repo",):
    if p not in sys.path:
        sys.path.insert(0, p)

from contextlib import ExitStack

import numpy as np

import concourse.bacc as bacc
import concourse.bass as bass
import concourse.mybir as mybir
import concourse.tile as tile
from concourse.masks import make_identity

F32 = mybir.dt.float32
AF = mybir.ActivationFunctionType

B, T, D, H, O = 256, 256, 256, 1024, 256
NCORES = 8
BL = B // NCORES  # 32
G4 = 4 * H  # 4096
KX, KY, KH = D // 128, O // 128, H // 128  # 2, 2, 8


def gate_perm() -> np.ndarray:
    """Map stored z column position -> original gate column (i,f,g,o order)."""
    perm = np.empty(G4, dtype=np.int64)
    for beta in (0, 1):
        for j in range(4):
            for half in (0, 1):
                gate = (0, 1, 2, 3)[2 * beta + half]
                src = 1024 * gate + 256 * j
                pos = 2048 * beta + 512 * j + 256 * half
                perm[pos : pos + 256] = np.arange(src, src + 256)
    return perm


def _hT_off(c: int) -> int:
    """Column offset of h^T chunk c (channels 128c:128c+128) inside hT_sb."""
    return 128 * (c % 2) + 32 * (c // 2)


def round_f32r(a: np.ndarray) -> np.ndarray:
    """Round-to-nearest-even fp32 -> fp32r (low 12 mantissa bits zeroed)."""
    u = np.ascontiguousarray(a, dtype=np.float32).view(np.uint32)
    lsb = (u >> np.uint32(12)) & np.uint32(1)
    u = (u + np.uint32(0x7FF) + lsb) & np.uint32(0xFFFFF000)
    return u.view(np.float32)


FUNNEL = False

# Validated precision/speed split (2026-08-08): Wh streamed as fp8e3m4 with
# x16 z-path pre-scaling halves the dominant SBUF->PE weight traffic;
# measured rel err 1.58e-2 (< 2e-2 gate) vs 7.77e-3 all-bf16.
W8 = ("Wh",)
W8_E3 = True
ZSCALE = 16.0


def build_nc(T_steps: int = T, use_bias_z: bool = False, use_bias_y: bool = False,
             mm_dt=mybir.dt.bfloat16, reps: int = 1, mm_only: int = 0,
             y_pack: bool = False, w8: tuple = W8, w8_e3: bool = W8_E3,
             zscale: float = ZSCALE):
    """reps>1 repeats the ENTIRE kernel body (weight loads + recurrence)
    inside one NEFF. Every rep recomputes the same output from the same
    inputs, so correctness is unchanged; (wall(reps=a)-wall(reps=b))/(a-b)
    isolates one full kernel execution from dispatch/transfer overhead.

    mm_only (diagnostics; output garbage): 1 = emit only the PE matmul
    stream (no gate chain / feedback / output) with the normal 4-way
    tile_position column packing; 2 = same but all matmuls forced to
    tile_position (0,0) (no column packing).

    y_pack: compute y = h @ Wd with 4-way tile_position column packing
    (2 K-chunks per column group -> 4 partition-block partials in PSUM)
    followed by a DVE reduction, instead of 8 serial matmuls.

    w8: subset of ("Wxx","Wxy","Wh","Wd") to store + stream as fp8
    (activations stay bf16). Halves the SBUF->PE weight-stream bytes for
    those tensors. w8_e3 selects fp8e3m4 (4 mantissa bits, ~1.35% rms with
    zscale=16 pre-scaling) instead of e4m3 (~2.7% rms). zscale>1: the host
    pre-multiplies all THREE z-path weights (Wxx/Wxy/Wh) by zscale (exact
    for bf16) so e3m4's narrow exponent range is centered; the gate
    activations fold in scale=1/zscale. Wd is never scaled."""
    nc = bacc.Bacc()
    F8 = mybir.dt.float8e3 if w8_e3 else mybir.dt.float8e4
    zs = 1.0 / zscale
    assert not (use_bias_z and zscale != 1.0), "bias_z would need zscale too"
    assert "Wd" not in w8 or zscale == 1.0, "Wd is unscaled"


    def wdt(name):
        return F8 if name in w8 else mm_dt

    xT_d = nc.declare_dram_parameter("xT", [T_steps, 128, 2 * BL], mm_dt,
                                     isOutput=False)
    Wxx_d = nc.declare_dram_parameter("Wxx", [D, G4], wdt("Wxx"), isOutput=False)
    Wxy_d = nc.declare_dram_parameter("Wxy", [O, G4], wdt("Wxy"), isOutput=False)
    Wh_d = nc.declare_dram_parameter("Wh", [H, G4], wdt("Wh"), isOutput=False)
    Wd_d = nc.declare_dram_parameter("Wd", [H, O], wdt("Wd"), isOutput=False)
    bz_d = by_d = None
    if use_bias_z:
        bz_d = nc.declare_dram_parameter("bz", [128, 1024], F32, isOutput=False)
    if use_bias_y:
        by_d = nc.declare_dram_parameter("by", [BL, O], F32, isOutput=False)
    ys_d = nc.declare_dram_parameter("ys", [BL, T_steps, O], F32, isOutput=True)

    def mc(ap):
        return ap.bitcast(mm_dt) if ap.dtype != mm_dt else ap

    with tile.TileContext(nc) as tc:
        with ExitStack() as ctx:
            wpool = ctx.enter_context(tc.tile_pool(name="weights", bufs=1))
            state = ctx.enter_context(tc.tile_pool(name="state", bufs=1))
            xpool = ctx.enter_context(tc.tile_pool(name="xin", bufs=3))
            gpool = ctx.enter_context(tc.tile_pool(name="gates", bufs=1))
            hpool = ctx.enter_context(tc.tile_pool(name="hT", bufs=1))
            ypool = ctx.enter_context(tc.tile_pool(name="yt", bufs=1))
            zpsum = ctx.enter_context(tc.tile_pool(name="zps", bufs=2, space="PSUM"))
            ypsum = ctx.enter_context(tc.tile_pool(name="yps", bufs=2, space="PSUM"))
            tpsum = ctx.enter_context(tc.tile_pool(name="tps", bufs=2, space="PSUM"))

            Wxx_sb = wpool.tile([128, KX * G4], wdt("Wxx"))
            Wxy_sb = wpool.tile([128, KY * G4], wdt("Wxy"))
            Wh_sb = wpool.tile([128, KH * G4], wdt("Wh"))
            Wd_sb = wpool.tile([128, KH * O], wdt("Wd"))
            if use_bias_z:
                bz_sb = wpool.tile([128, 1024], F32)
            if use_bias_y:
                by_sb = wpool.tile([BL, O], F32)

            # identity for PE transposes (bf16: f32 transpose-mode faults on
            # hw); I64 in both partition halves so the fmap can start at
            # partition 0 or 64 (must match the weights)
            ident = wpool.tile([128, 128], mm_dt)
            make_identity(nc, ident[:])
            nc.vector.tensor_copy(ident[:], ident[:])  # launder Pool dep -> DVE

            # c state, channel(32j+b, n) = 256j + n
            c_sb = state.tile([128, 256], F32)
            if mm_only:
                # bf16 dummy feedback operand (weight tiles may be fp8)
                dum_sb = state.tile([128, 256], mm_dt)
                nc.gpsimd.memset(dum_sb[:], 0.0)
                nc.vector.tensor_copy(dum_sb[:], dum_sb[:])

            def load_weights():
                # Matmult instructions can carry at most ONE sem wait in this
                # lowering; every matmul dependency must resolve to a single
                # DVE sem value. Weight DMAs are therefore "laundered" through
                # in-place DVE copies (one per DMA so each copy waits on one
                # DMA-queue sem only).
                for k in range(KX):
                    nc.sync.dma_start(Wxx_sb[:, k * G4 : (k + 1) * G4],
                                      Wxx_d[k * 128 : (k + 1) * 128, :])
                    nc.vector.tensor_copy(Wxx_sb[:, k * G4 : (k + 1) * G4],
                                          Wxx_sb[:, k * G4 : (k + 1) * G4])
                for k in range(KY):
                    nc.sync.dma_start(Wxy_sb[:, k * G4 : (k + 1) * G4],
                                      Wxy_d[k * 128 : (k + 1) * 128, :])
                    nc.vector.tensor_copy(Wxy_sb[:, k * G4 : (k + 1) * G4],
                                          Wxy_sb[:, k * G4 : (k + 1) * G4])
                for k in range(KH):
                    nc.sync.dma_start(Wh_sb[:, k * G4 : (k + 1) * G4],
                                      Wh_d[k * 128 : (k + 1) * 128, :])
                    nc.vector.tensor_copy(Wh_sb[:, k * G4 : (k + 1) * G4],
                                          Wh_sb[:, k * G4 : (k + 1) * G4])
                    nc.sync.dma_start(Wd_sb[:, k * O : (k + 1) * O],
                                      Wd_d[k * 128 : (k + 1) * 128, :])
                    nc.vector.tensor_copy(Wd_sb[:, k * O : (k + 1) * O],
                                          Wd_sb[:, k * O : (k + 1) * O])
                if use_bias_z:
                    nc.sync.dma_start(bz_sb[:], bz_d[:, :])
                if use_bias_y:
                    nc.sync.dma_start(by_sb[:], by_d[:, :])

            def emit_z_mms(z_tile, chunks, start, stop):
                nck = len(chunks)
                for ci, (lhsT, wtile, coff) in enumerate(chunks):
                    for beta in range(2):
                        for j in range(4):
                            w_lo = coff + 2048 * beta + 512 * j
                            jo = 0 if mm_only == 2 else j
                            nc.tensor.matmul(
                                z_tile[32 * jo : 32 * (jo + 1),
                                       512 * beta : 512 * (beta + 1)],
                                mc(lhsT),
                                wtile[:, w_lo : w_lo + 512],
                                start=(start and ci == 0),
                                stop=(stop and ci == nck - 1),
                                tile_position=(0, 32 * jo),
                                skip_group_check=True,
                            )

            def load_x(t):
                xT_sb = xpool.tile([128, 2 * BL], mm_dt, name="xT_sb")
                nc.sync.dma_start(xT_sb[:], xT_d[t])
                # launder the x DMA-queue sem into the DVE sem
                xr_sb = xpool.tile([128, 2 * BL], mm_dt, name="xr_sb")
                nc.vector.tensor_copy(xr_sb[:], xT_sb[:])
                return [(xr_sb[:, bass.ts(k, BL)], Wxx_sb, k * G4)
                        for k in range(KX)]

            for _rep in range(reps):
              load_weights()
              nc.gpsimd.memset(c_sb[:], 0.0)
              hT_prev = None
              yT_prev = None
              # software pipeline: the x-part of step t+1 is issued during
              # step t, so the in-order PE has independent work while the gate
              # chain (ACT/DVE) of step t runs.
              z_ps = zpsum.tile([128, 1024], F32, name="z_ps")
              emit_z_mms(z_ps, load_x(0), start=True, stop=(T_steps == 1))
              for t in range(T_steps):
                if t > 0:
                    # h first, y last: the y feedback chain (Wd+tanh+cast+
                    # transpose) of step t-1 gets the h-matmul span as slack
                    chunks = [(hT_prev[:, _hT_off(k) : _hT_off(k) + BL], Wh_sb,
                               k * G4) for k in (0, 2, 4, 6, 1, 3, 5, 7)]
                    chunks += [(yT_prev[:, bass.ts(k, BL)], Wxy_sb, k * G4)
                               for k in range(KY)]
                    emit_z_mms(z_ps, chunks, start=False, stop=True)
                if t + 1 < T_steps:
                    z_next = zpsum.tile([128, 1024], F32, name="z_ps")
                    emit_z_mms(z_next, load_x(t + 1), start=True, stop=False)
                else:
                    z_next = None

                if mm_only:
                    # PE stream only: dummy feedback operands, no gate chain
                    y_ps = ypsum.tile([BL, O], F32, name="y_ps")
                    for k in range(KH):
                        nc.tensor.matmul(
                            y_ps[:], dum_sb[:, 0:BL],
                            Wd_sb[:, k * O : (k + 1) * O],
                            start=(k == 0), stop=(k == KH - 1),
                        )
                    hT_prev = dum_sb
                    yT_prev = dum_sb
                    z_ps = z_next
                    continue

                # gate math: <=1 PSUM operand per DVE op
                if use_bias_z:
                    nc.vector.tensor_add(z_ps[:, 0:512], z_ps[:, 0:512],
                                         bz_sb[:, 0:512])
                    nc.vector.tensor_add(z_ps[:, 512:1024], z_ps[:, 512:1024],
                                         bz_sb[:, 512:1024])
                # gate chain split into column halves: half 0 finishes ->
                # its transpose + hT copy run while half 1 still computes, so
                # the even hT-chunk matmuls of step t+1 start earlier
                tg_sb = gpool.tile([128, 256], F32, name="tg_sb")
                o_sb = gpool.tile([128, 256], F32, name="o_sb")
                h_stk = gpool.tile([128, 256], mm_dt, name="h_stk")
                tr_ps = tpsum.tile([128, 320], mm_dt, name="tr_ps")
                hT_sb = hpool.tile([128, 256], mm_dt, name="hT_sb")
                for hf in range(2):
                    ve = nc.vector
                    s = slice(128 * hf, 128 * hf + 128)
                    nc.scalar.activation(tg_sb[:, s], z_ps[:, 512 + 128 * hf :
                                                           640 + 128 * hf],
                                         AF.Tanh, scale=zs)
                    nc.scalar.activation(z_ps[:, s], z_ps[:, s], AF.Sigmoid,
                                         scale=zs)
                    ve.tensor_mul(tg_sb[:, s], z_ps[:, s], tg_sb[:, s])
                    nc.scalar.activation(z_ps[:, 256 + 128 * hf : 384 + 128 * hf],
                                         z_ps[:, 256 + 128 * hf : 384 + 128 * hf],
                                         AF.Sigmoid, scale=zs)
                    ve.tensor_mul(c_sb[:, s],
                                  z_ps[:, 256 + 128 * hf : 384 + 128 * hf],
                                  c_sb[:, s])
                    nc.scalar.activation(o_sb[:, s], z_ps[:, 768 + 128 * hf :
                                                          896 + 128 * hf],
                                         AF.Sigmoid, scale=zs)
                    ve.tensor_add(c_sb[:, s], tg_sb[:, s], c_sb[:, s])
                    nc.scalar.activation(tg_sb[:, s], c_sb[:, s], AF.Tanh)
                    ve.tensor_mul(h_stk[:, s], o_sb[:, s], tg_sb[:, s])
                    nc.tensor.transpose(tr_ps[:, s], h_stk[:, s], ident[:])
                    ve.tensor_copy(hT_sb[:, s], tr_ps[:, s])

                # y = tanh(h @ Wd + bd)
                if y_pack and not use_bias_y:
                    # 4 column groups x 2 K-chunks -> 4 partition-block
                    # partials, reduced on DVE (<=1 PSUM operand per op)
                    y4_ps = ypsum.tile([128, O], F32, name="y_ps")
                    for g in range(4):
                        for kk in range(2):
                            k = 2 * g + kk
                            nc.tensor.matmul(
                                y4_ps[32 * g : 32 * (g + 1), :],
                                mc(hT_sb[:, _hT_off(k) : _hT_off(k) + BL]),
                                Wd_sb[:, k * O : (k + 1) * O],
                                start=(kk == 0),
                                stop=(kk == 1),
                                tile_position=(0, 32 * g),
                                skip_group_check=True,
                            )
                    yr1 = gpool.tile([BL, O], F32, name="yr1")
                    yr2 = gpool.tile([BL, O], F32, name="yr2")
                    nc.vector.tensor_copy(yr1[:], y4_ps[32:64, :])
                    nc.vector.tensor_add(yr1[:], y4_ps[0:32, :], yr1[:])
                    nc.vector.tensor_copy(yr2[:], y4_ps[96:128, :])
                    nc.vector.tensor_add(yr2[:], y4_ps[64:96, :], yr2[:])
                    nc.vector.tensor_add(yr1[:], yr1[:], yr2[:])
                    y_src = yr1
                else:
                    y_ps = ypsum.tile([BL, O], F32, name="y_ps")
                    if FUNNEL:
                        nc.vector.tensor_copy(y_ps[:], Wxx_sb[0:BL, 0:256])
                    for k in range(KH):
                        nc.tensor.matmul(
                            y_ps[:],
                            mc(hT_sb[:, _hT_off(k) : _hT_off(k) + BL]),
                            Wd_sb[:, k * O : (k + 1) * O],
                            start=(k == 0),
                            stop=(k == KH - 1),
                        )
                    if use_bias_y:
                        nc.vector.tensor_add(y_ps[:], y_ps[:], by_sb[:])
                    y_src = y_ps
                # own double-buffered tile so the output DMA never blocks
                # the next step's gate ACTs
                y_sb = ypool.tile([BL, O], F32, name="y_sb", bufs=2)
                nc.scalar.activation(y_sb[:], y_src[:], AF.Tanh)
                nc.sync.dma_start(ys_d[:, t, :], y_sb[:])
                # cast y for the bf16 PE-transposes (also launders ACT -> DVE)
                y_bf = ypool.tile([BL, O], mm_dt, name="y_bf")
                nc.vector.tensor_copy(y_bf[:], y_sb[:])

                # y -> yT via 2 PE transposes
                for q in range(2):
                    nc.tensor.transpose(
                        tr_ps[:, 256 + 32 * q : 256 + 32 * (q + 1)],
                        y_bf[0:BL, 128 * q : 128 * (q + 1)],
                        ident[0:32, 0:32],
                    )
                yT_sb = ypool.tile([128, 2 * BL], mm_dt, name="yT_sb")
                nc.vector.tensor_copy(yT_sb[:], tr_ps[:, 256:320])

                hT_prev = hT_sb
                yT_prev = yT_sb
                z_ps = z_next

    nc.compile()
    return nc


def prep_inputs(x, Wx, Wh, b, Wd, bd, T_steps: int = T, w8: tuple = W8,
                w8_e3: bool = W8_E3, zscale: float = ZSCALE):
    """Host-side shard + relayout. Returns (in_maps, use_bias_z, use_bias_y)."""
    x = np.asarray(x, dtype=np.float32)[:, :T_steps, :]
    Wx = np.asarray(Wx, dtype=np.float32)
    Wh = np.asarray(Wh, dtype=np.float32)
    b = np.asarray(b, dtype=np.float32)
    Wd = np.asarray(Wd, dtype=np.float32)
    bd = np.asarray(bd, dtype=np.float32)

    import ml_dtypes

    f8np = mybir.dt.np(mybir.dt.float8e3 if w8_e3 else mybir.dt.float8e4)

    def wcast(name, arr):
        dt = f8np if name in w8 else ml_dtypes.bfloat16
        if name != "Wd":
            arr = arr * np.float32(zscale)
        return np.ascontiguousarray(arr).astype(dt)

    perm = gate_perm()
    Wxp = np.ascontiguousarray(Wx[:, perm])
    Whp_f = np.ascontiguousarray(Wh[:, perm])
    Wxx = wcast("Wxx", Wxp[:D])
    Wxy = wcast("Wxy", Wxp[D:])
    Whp = wcast("Wh", Whp_f)
    Wd = wcast("Wd", Wd)

    use_bias_z = bool(np.any(b))
    use_bias_y = bool(np.any(bd))
    shared = {"Wxx": Wxx, "Wxy": Wxy, "Wh": Whp, "Wd": Wd}
    if use_bias_z:
        bp = b[perm]
        bz = np.empty((128, 1024), dtype=np.float32)
        for j in range(4):
            for beta in range(2):
                bz[32 * j : 32 * (j + 1), 512 * beta : 512 * (beta + 1)] = bp[
                    2048 * beta + 512 * j : 2048 * beta + 512 * j + 512][None, :]
        shared["bz"] = bz
    if use_bias_y:
        shared["by"] = np.broadcast_to(bd, (BL, O)).copy()

    in_maps = []
    for c in range(NCORES):
        xc = x[c * BL : (c + 1) * BL]                      # [BL, T, D]
        xT = xc.transpose(1, 2, 0)                         # [T, D, BL]
        xT = xT.reshape(T_steps, 2, 128, BL).transpose(0, 2, 1, 3)
        import ml_dtypes
        xT = np.ascontiguousarray(
            xT.reshape(T_steps, 128, 2 * BL)).astype(ml_dtypes.bfloat16)
        in_maps.append({"xT": xT, **shared})
    return in_maps, use_bias_z, use_bias_y


def kernel(x, Wx, Wh, b, Wd, bd):
    from concourse.bass_utils import run_bass_kernel_spmd

    in_maps, ubz, uby = prep_inputs(x, Wx, Wh, b, Wd, bd, T)
    nc = build_nc(T, ubz, uby)
    res = run_bass_kernel_spmd(nc, in_maps, list(range(NCORES)))
    ys = np.concatenate([res.results[c]["ys"] for c in range(NCORES)], axis=0)
    return ys.astype(np.float32)

